# revision 10
# baseline (speedup 1.0000x reference)
"""Multi-head attention (b=2, n=2048, dim=1024, 16 heads x 64) on 8 trn2 cores.

Sharding: core c owns (batch b=c//4, heads 4i..4i+3 where i=c%4).
 - Each core computes q^T,k^T (feature-major, head-pair stacked on partitions)
   and v (token-major, with an appended ones-column per head that makes the
   softmax denominators fall out of the o-matmul for free) for its 4 heads
   over its batch's 2048 tokens, from host-pretransposed x^T.
 - Attention per head: simT[k_tok, q_tok] matmuls, exp on ACT (no max
   subtraction: sim ~ N(0,1), fp32 exp is exact enough), o^T accumulated on
   PE, then per-q normalization via reciprocal + gpsimd partition-broadcast.
 - Normalized per-head o^T rows are AllGathered (2 groups of 4 = per batch) so
   every core holds att^T [1024 feat, 2048 tok]; each core then computes a
   dim/4-wide output-dim shard of out = att @ w_out.
All matmuls run as float32r (fp32 data, FP22 multiply) for 4x PE throughput.
"""

import os
import sys

for _p in ("/opt/trn_rl_repo", "/root/.axon_site/_ro/trn_rl_repo"):
    if os.path.isdir(_p) and _p not in sys.path:
        sys.path.insert(0, _p)

import numpy as np

import concourse.bass as bass
import concourse.mybir as mybir
import concourse.tile as tile
from concourse import bass_utils, library_config

F32 = mybir.dt.float32
F32R = mybir.dt.float32r
N_CORES = 8
HEADS_TOTAL = 16
DH = 64
FEAT = HEADS_TOTAL * DH  # 1024 attention feature dim (fixed by the module)

_MAX_WAITS = 1  # walrus in this container encodes at most 1 sem wait per inst


def _split_excess_waits(nc):
    for f in nc.m.functions:
        for bb in f.blocks:
            new = []
            for inst in bb.instructions:
                si = inst.sync_info
                if si is not None and si.on_wait and len(si.on_wait) > _MAX_WAITS:
                    waits = list(si.on_wait)
                    keep = waits[-_MAX_WAITS:]
                    extra = waits[:-_MAX_WAITS]
                    for j in range(0, len(extra), _MAX_WAITS):
                        new.append(
                            mybir.InstNoOp(
                                name=f"{inst.name}-waitsplit-{j}",
                                engine=inst.engine,
                                ins=[],
                                outs=[],
                                sync_info=mybir.SyncInfo(
                                    on_wait=extra[j : j + _MAX_WAITS], on_update=[]
                                ),
                            )
                        )
                    si.on_wait = keep
                new.append(inst)
            bb.instructions[:] = new


def build_nc(dim=1024, tok=2048, split_waits=True):
    """One SPMD program; all 8 cores run it on different input shards."""
    assert dim % 128 == 0 and tok % 128 == 0
    KC = dim // 128          # contraction chunks over model dim
    TT = tok // 128          # token tiles (k-side)
    QCH = min(512, tok)      # q-chunk width
    NQC = tok // QCH
    HPC = HEADS_TOTAL // (N_CORES // 2)  # heads per core = 4
    QF = HPC * DH            # 256 q/k features per core
    QM = QF // 128           # 2 feature chunks (head pairs)
    VW = HPC * (DH + 1)      # 260: v columns + per-head ones column
    NOUT = dim // 4          # out-dim shard per core (4 shards per batch)
    FC = FEAT // 128         # 8 att^T feature chunks after the gather

    nc = bass.Bass("TRN2", target_bir_lowering=False, debug=False,
                   num_devices=N_CORES)

    xT = nc.dram_tensor("xT", [dim, tok], F32, kind="ExternalInput").ap()
    wq = nc.dram_tensor("wq", [dim, QF], F32, kind="ExternalInput").ap()
    wk = nc.dram_tensor("wk", [dim, QF], F32, kind="ExternalInput").ap()
    wv = nc.dram_tensor("wv", [dim, VW], F32, kind="ExternalInput").ap()
    wvi = nc.dram_tensor("wvi", [1, VW], F32, kind="ExternalInput").ap()
    ones = nc.dram_tensor("ones", [1, 128], F32, kind="ExternalInput").ap()
    wout = nc.dram_tensor("wout", [FEAT, NOUT], F32, kind="ExternalInput").ap()
    out = nc.dram_tensor("out", [tok, NOUT], F32, kind="ExternalOutput").ap()
    ag_in = nc.dram_tensor("ag_in", [QF, tok], F32).ap()
    ag_out = nc.dram_tensor("ag_out", [FEAT, tok], F32).ap()

    with tile.TileContext(nc) as tc:
        with (
            tc.tile_pool(name="big", bufs=max(KC, FC)) as big,
            tc.tile_pool(name="wpool", bufs=1) as wpool,
            tc.tile_pool(name="work", bufs=1) as work,
            tc.tile_pool(name="stage", bufs=3) as stage,
            tc.tile_pool(name="psum", bufs=1, space="PSUM") as psum,
        ):
            # ---- load inputs -------------------------------------------------
            xT_sb = []
            for k in range(KC):
                xt = big.tile([128, tok], F32R, name=f"xT{k}", tag="big")
                nc.sync.dma_start(xt[:], xT[128 * k : 128 * (k + 1), :].bitcast(F32R))
                xT_sb.append(xt)

            wq_sb = wpool.tile([128, KC, QF], F32R, name="wq_sb")
            nc.sync.dma_start(wq_sb[:], wq.rearrange("(k p) m -> p k m", p=128).bitcast(F32R))
            wk_sb = wpool.tile([128, KC, QF], F32R, name="wk_sb")
            nc.sync.dma_start(wk_sb[:], wk.rearrange("(k p) m -> p k m", p=128).bitcast(F32R))
            wv_sb = wpool.tile([128, KC, VW], F32R, name="wv_sb")
            nc.sync.dma_start(wv_sb[:], wv.rearrange("(k p) m -> p k m", p=128).bitcast(F32R))
            wvi_sb = wpool.tile([1, VW], F32R, name="wvi_sb")
            nc.sync.dma_start(wvi_sb[:], wvi[:].bitcast(F32R))
            wout_sb = wpool.tile([128, FC, NOUT], F32R, name="wout_sb")
            nc.sync.dma_start(wout_sb[:], wout.rearrange("(k p) m -> p k m", p=128).bitcast(F32R))
            ones_sb = wpool.tile([1, 128], F32R, name="ones_sb")
            nc.sync.dma_start(ones_sb[:], ones[:].bitcast(F32R))

            kT_sb = work.tile([128, QM, tok], F32R, name="kT_sb")
            qT_sb = work.tile([128, QM, tok], F32R, name="qT_sb")
            v_sb = work.tile([128, TT, VW], F32R, name="v_sb")
            oT_sb = work.tile([128, QM, tok], F32, name="oT_sb")

            # ---- k^T, q^T : [feat, tok], head-pair-stacked on partitions ----
            for wsb, dst in ((wk_sb, kT_sb), (wq_sb, qT_sb)):
                for m in range(QM):
                    for qc in range(NQC):
                        ps = psum.tile([128, 1024], F32, name="mmps", tag="sim",
                                       bufs=2)
                        for k in range(KC):
                            nc.tensor.matmul(
                                ps[:, :QCH],
                                wsb[:, k, 128 * m : 128 * (m + 1)],
                                xT_sb[k][:, QCH * qc : QCH * (qc + 1)],
                                start=(k == 0),
                                stop=(k == KC - 1),
                            )
                        nc.vector.tensor_copy(
                            dst[:, m, QCH * qc : QCH * (qc + 1)], ps[:, :QCH]
                        )

            # ---- v (token-major) with ones columns --------------------------
            for t in range(TT):
                ps = psum.tile([128, 1024], F32, name="vps", tag="sim", bufs=2)
                for k in range(KC):
                    nc.tensor.matmul(
                        ps[:, :VW],
                        xT_sb[k][:, 128 * t : 128 * (t + 1)],
                        wv_sb[:, k, :],
                        start=(k == 0),
                        stop=False,
                    )
                nc.tensor.matmul(  # ones columns via K=1 indicator matmul
                    ps[:, :VW],
                    ones_sb[:, :],
                    wvi_sb[:, :],
                    start=False,
                    stop=True,
                )
                nc.vector.tensor_copy(v_sb[:, t, :], ps[:, :VW])

            # ---- attention ---------------------------------------------------
            for qc in range(NQC):
                o_all = [
                    psum.tile([128, 512], F32, name=f"ops{h}", tag="o", bufs=4)
                    for h in range(2 * QM)
                ]
                for p in range(QM):
                    o_ps = o_all[2 * p : 2 * p + 2]
                    for j in range(TT):
                        sim = psum.tile([128, 1024], F32, name="sim", tag="sim",
                                        bufs=2)
                        ex = stage.tile([128, 1024], F32R, name="ex", tag="ex")
                        for hh in range(2):
                            nc.tensor.matmul(
                                sim[:, 512 * hh : 512 * hh + QCH],
                                kT_sb[64 * hh : 64 * hh + 64, p,
                                      128 * j : 128 * (j + 1)],
                                qT_sb[64 * hh : 64 * hh + 64, p,
                                      QCH * qc : QCH * (qc + 1)],
                                start=True, stop=True,
                            )
                        for hh in range(2):
                            nc.scalar.activation(
                                ex[:, 512 * hh : 512 * hh + QCH],
                                sim[:, 512 * hh : 512 * hh + QCH],
                                mybir.ActivationFunctionType.Exp,
                            )
                        for hh in range(2):
                            h = 2 * p + hh
                            nc.tensor.matmul(
                                o_ps[hh][: DH + 1, :QCH],
                                v_sb[:, j, 65 * h : 65 * h + DH + 1],
                                ex[:, 512 * hh : 512 * hh + QCH],
                                start=(j == 0),
                                stop=(j == TT - 1),
                            )
                for p in range(QM):
                    for hh in range(2):
                        h = 2 * p + hh
                        r = stage.tile([1, QCH], F32R, name="r", tag="r", bufs=2)
                        with nc.allow_low_precision(reason="f32r recip for bcast mm"):
                            nc.vector.reciprocal(r[:], o_all[h][DH : DH + 1, :QCH])
                        rb_ps = psum.tile([128, 1024], F32, name="rbps", tag="sim",
                                          bufs=2)
                        nc.tensor.matmul(
                            rb_ps[:64, :QCH], ones_sb[:, :64], r[:],
                            start=True, stop=True,
                        )
                        rb = stage.tile([64, QCH], F32, name="rb", tag="rb", bufs=2)
                        nc.vector.tensor_copy(rb[:], rb_ps[:64, :QCH])
                        nc.vector.tensor_mul(
                            oT_sb[64 * hh : 64 * hh + 64, p,
                                  QCH * qc : QCH * (qc + 1)],
                            o_all[h][:DH, :QCH],
                            rb[:],
                        )
            # ---- exchange heads across the batch group ----------------------
            for m in range(QM):
                nc.sync.dma_start(ag_in[128 * m : 128 * (m + 1), :],
                                  oT_sb[:, m, :])
            nc.gpsimd.collective_compute(
                "AllGather",
                mybir.AluOpType.bypass,
                replica_groups=[[0, 1, 2, 3], [4, 5, 6, 7]],
                ins=[ag_in[:]],
                outs=[ag_out[:]],
            )

            # ---- out projection: out[:, shard] = att @ wout -----------------
            att_sb = []
            for k in range(FC):
                at = big.tile([128, tok], F32R, name=f"att{k}", tag="big")
                nc.sync.dma_start(at[:], ag_out[128 * k : 128 * (k + 1), :].bitcast(F32R))
                att_sb.append(at)
            for t in range(TT):
                ps = psum.tile([128, 1024], F32, name="outps", tag="sim", bufs=2)
                for k in range(FC):
                    nc.tensor.matmul(
                        ps[:, :NOUT],
                        att_sb[k][:, 128 * t : 128 * (t + 1)],
                        wout_sb[:, k, :],
                        start=(k == 0),
                        stop=(k == FC - 1),
                    )
                os_ = stage.tile([128, NOUT], F32, name="os", tag="os", bufs=2)
                nc.vector.tensor_copy(os_[:], ps[:, :NOUT])
                nc.sync.dma_start(out[128 * t : 128 * (t + 1), :], os_[:])

    if split_waits:
        _split_excess_waits(nc)
    return nc


_NC_CACHE = {}


def _get_nc(dim, tok):
    key = (dim, tok)
    if key not in _NC_CACHE:
        _NC_CACHE[key] = build_nc(dim, tok)
    return _NC_CACHE[key]


def make_in_maps(x, w_qkv, w_out):
    """Host-side sharding/layout prep. x:[2,n,dim] w_qkv:[dim,3*h*dh] w_out:[h*dh,dim]."""
    b, n, dim = x.shape
    scale = DH ** -0.5
    hpc = HEADS_TOTAL // (N_CORES // 2)
    in_maps = []
    xT_b = [np.ascontiguousarray(x[bb].T) for bb in range(b)]
    nout = w_out.shape[1] // 4
    for c in range(N_CORES):
        bb, i = divmod(c, N_CORES // 2)
        heads = range(hpc * i, hpc * (i + 1))
        wq = np.concatenate(
            [w_qkv[:, DH * h : DH * (h + 1)] for h in heads], axis=1
        ) * np.float32(scale)
        wk = np.concatenate(
            [w_qkv[:, FEAT + DH * h : FEAT + DH * (h + 1)] for h in heads], axis=1
        )
        vw = hpc * (DH + 1)
        wv = np.zeros((dim, vw), np.float32)
        wvi = np.zeros((1, vw), np.float32)
        for j, h in enumerate(heads):
            wv[:, 65 * j : 65 * j + DH] = w_qkv[
                :, 2 * FEAT + DH * h : 2 * FEAT + DH * (h + 1)
            ]
            wvi[0, 65 * j + DH] = 1.0
        wo = np.ascontiguousarray(w_out[:, nout * i : nout * (i + 1)])
        in_maps.append({
            "xT": xT_b[bb],
            "wq": np.ascontiguousarray(wq, dtype=np.float32),
            "wk": np.ascontiguousarray(wk, dtype=np.float32),
            "wv": wv,
            "wvi": wvi,
            "ones": np.ones((1, 128), np.float32),
            "wout": wo.astype(np.float32),
        })
    return in_maps


def kernel(x, w_qkv, w_out):
    x = np.asarray(x, np.float32)
    w_qkv = np.asarray(w_qkv, np.float32)
    w_out = np.asarray(w_out, np.float32)
    b, n, dim = x.shape
    nc = _get_nc(dim, n)
    in_maps = make_in_maps(x, w_qkv, w_out)
    res = bass_utils.run_bass_kernel_spmd(nc, in_maps, core_ids=list(range(N_CORES)))
    dout = w_out.shape[1]
    nout = dout // 4
    full = np.empty((b, n, dout), np.float32)
    for c in range(N_CORES):
        bb, i = divmod(c, N_CORES // 2)
        full[bb, :, nout * i : nout * (i + 1)] = res.results[c]["out"]
    return full


# revision 13
# speedup vs baseline: 1.0760x; 1.0760x over previous
"""Multi-head attention (b=2, n=2048, dim=1024, 16 heads x 64) on 8 trn2 cores.

Sharding: core c owns (batch b=c//4, heads 4i..4i+3 where i=c%4).
 - Each core computes q^T,k^T (feature-major, head-pair stacked on partitions)
   and v (token-major, with an appended ones-column per head that makes the
   softmax denominators fall out of the o-matmul for free) for its 4 heads
   over its batch's 2048 tokens, from host-pretransposed x^T.
 - Attention per head: simT[k_tok, q_tok] matmuls, exp on ACT (no max
   subtraction: sim ~ N(0,1), fp32 exp is exact enough), o^T accumulated on
   PE, then per-q normalization via reciprocal + gpsimd partition-broadcast.
 - Normalized per-head o^T rows are AllGathered (2 groups of 4 = per batch) so
   every core holds att^T [1024 feat, 2048 tok]; each core then computes a
   dim/4-wide output-dim shard of out = att @ w_out.
All matmuls run as float32r (fp32 data, FP22 multiply) for 4x PE throughput.
"""

import os
import sys

for _p in ("/opt/trn_rl_repo", "/root/.axon_site/_ro/trn_rl_repo"):
    if os.path.isdir(_p) and _p not in sys.path:
        sys.path.insert(0, _p)

import numpy as np

import concourse.bass as bass
import concourse.mybir as mybir
import concourse.tile as tile
from concourse import bass_utils, library_config

F32 = mybir.dt.float32
F32R = mybir.dt.float32r
N_CORES = 8
HEADS_TOTAL = 16
DH = 64
FEAT = HEADS_TOTAL * DH  # 1024 attention feature dim (fixed by the module)

_MAX_WAITS = 1  # walrus in this container encodes at most 1 sem wait per inst


def _split_excess_waits(nc):
    for f in nc.m.functions:
        for bb in f.blocks:
            new = []
            for inst in bb.instructions:
                si = inst.sync_info
                if si is not None and si.on_wait and len(si.on_wait) > _MAX_WAITS:
                    waits = list(si.on_wait)
                    keep = waits[-_MAX_WAITS:]
                    extra = waits[:-_MAX_WAITS]
                    for j in range(0, len(extra), _MAX_WAITS):
                        new.append(
                            mybir.InstNoOp(
                                name=f"{inst.name}-waitsplit-{j}",
                                engine=inst.engine,
                                ins=[],
                                outs=[],
                                sync_info=mybir.SyncInfo(
                                    on_wait=extra[j : j + _MAX_WAITS], on_update=[]
                                ),
                            )
                        )
                    si.on_wait = keep
                new.append(inst)
            bb.instructions[:] = new


def build_nc(dim=1024, tok=2048, split_waits=True):
    """One SPMD program; all 8 cores run it on different input shards."""
    assert dim % 128 == 0 and tok % 128 == 0
    KC = dim // 128          # contraction chunks over model dim
    TT = tok // 128          # token tiles (k-side)
    QCH = min(512, tok)      # q-chunk width
    NQC = tok // QCH
    HPC = HEADS_TOTAL // (N_CORES // 2)  # heads per core = 4
    QF = HPC * DH            # 256 q/k features per core
    QM = QF // 128           # 2 feature chunks (head pairs)
    VW = HPC * (DH + 1)      # 260: v columns + per-head ones column
    NOUT = dim // 4          # out-dim shard per core (4 shards per batch)
    FC = FEAT // 128         # 8 att^T feature chunks after the gather

    nc = bass.Bass("TRN2", target_bir_lowering=False, debug=False,
                   num_devices=N_CORES)

    xT = nc.dram_tensor("xT", [dim, tok], F32, kind="ExternalInput").ap()
    wq = nc.dram_tensor("wq", [dim, QF], F32, kind="ExternalInput").ap()
    wk = nc.dram_tensor("wk", [dim, QF], F32, kind="ExternalInput").ap()
    wv = nc.dram_tensor("wv", [dim, VW], F32, kind="ExternalInput").ap()
    wvi = nc.dram_tensor("wvi", [1, VW], F32, kind="ExternalInput").ap()
    ones = nc.dram_tensor("ones", [1, 128], F32, kind="ExternalInput").ap()
    wout = nc.dram_tensor("wout", [FEAT, NOUT], F32, kind="ExternalInput").ap()
    out = nc.dram_tensor("out", [tok, NOUT], F32, kind="ExternalOutput").ap()
    SPLIT = 2 if NQC % 2 == 0 else 1
    HTOK = tok // SPLIT
    ag_in_h = [nc.dram_tensor(f"ag_in{s}", [QF, HTOK], F32).ap()
               for s in range(SPLIT)]
    ag_out_h = [nc.dram_tensor(f"ag_out{s}", [FEAT, HTOK], F32).ap()
                for s in range(SPLIT)]

    with tile.TileContext(nc) as tc:
        with (
            tc.tile_pool(name="big", bufs=max(KC, FC)) as big,
            tc.tile_pool(name="wpool", bufs=1) as wpool,
            tc.tile_pool(name="work", bufs=1) as work,
            tc.tile_pool(name="stage", bufs=3) as stage,
            tc.tile_pool(name="psum", bufs=1, space="PSUM") as psum,
        ):
            # ---- load inputs -------------------------------------------------
            xT_sb = []
            for k in range(KC):
                xt = big.tile([128, tok], F32R, name=f"xT{k}", tag="big")
                nc.sync.dma_start(xt[:], xT[128 * k : 128 * (k + 1), :].bitcast(F32R))
                xT_sb.append(xt)

            wq_sb = wpool.tile([128, KC, QF], F32R, name="wq_sb")
            nc.sync.dma_start(wq_sb[:], wq.rearrange("(k p) m -> p k m", p=128).bitcast(F32R))
            wk_sb = wpool.tile([128, KC, QF], F32R, name="wk_sb")
            nc.sync.dma_start(wk_sb[:], wk.rearrange("(k p) m -> p k m", p=128).bitcast(F32R))
            wv_sb = wpool.tile([128, KC, VW], F32R, name="wv_sb")
            nc.sync.dma_start(wv_sb[:], wv.rearrange("(k p) m -> p k m", p=128).bitcast(F32R))
            wvi_sb = wpool.tile([1, VW], F32R, name="wvi_sb")
            nc.sync.dma_start(wvi_sb[:], wvi[:].bitcast(F32R))
            wout_sb = wpool.tile([128, FC, NOUT], F32R, name="wout_sb")
            nc.sync.dma_start(wout_sb[:], wout.rearrange("(k p) m -> p k m", p=128).bitcast(F32R))
            ones_sb = wpool.tile([1, 128], F32R, name="ones_sb")
            nc.sync.dma_start(ones_sb[:], ones[:].bitcast(F32R))

            kT_sb = work.tile([128, QM, tok], F32R, name="kT_sb")
            qT_sb = work.tile([128, QM, tok], F32R, name="qT_sb")
            v_sb = work.tile([128, TT, VW], F32R, name="v_sb")
            oT_sb = work.tile([128, QM, tok], F32, name="oT_sb")

            # ---- q^T/k^T [feat, tok] head-pair-stacked; v token-major ------
            # Emission order: kT(m=0), all v, qT(m=0), then remaining chunks,
            # so the first head-pair's attention (and ACT) can start early.
            def qk_group(wsb, dst, m):
                for qc in range(NQC):
                    ps = psum.tile([128, 512], F32, name="mmps", tag="sim",
                                   bufs=4)
                    for k in range(KC):
                        nc.tensor.matmul(
                            ps[:, :QCH],
                            wsb[:, k, 128 * m : 128 * (m + 1)],
                            xT_sb[k][:, QCH * qc : QCH * (qc + 1)],
                            start=(k == 0),
                            stop=(k == KC - 1),
                        )
                    nc.vector.tensor_copy(
                        dst[:, m, QCH * qc : QCH * (qc + 1)], ps[:, :QCH]
                    )

            def v_group(t):
                ps = psum.tile([128, 512], F32, name="vps", tag="sim", bufs=4)
                for k in range(KC):
                    nc.tensor.matmul(
                        ps[:, :VW],
                        xT_sb[k][:, 128 * t : 128 * (t + 1)],
                        wv_sb[:, k, :],
                        start=(k == 0),
                        stop=False,
                    )
                nc.tensor.matmul(  # ones columns via K=1 indicator matmul
                    ps[:, :VW], ones_sb[:, :], wvi_sb[:, :],
                    start=False, stop=True,
                )
                nc.vector.tensor_copy(v_sb[:, t, :], ps[:, :VW])

            qk_group(wk_sb, kT_sb, 0)
            for t in range(TT):
                v_group(t)
            qk_group(wq_sb, qT_sb, 0)
            for m in range(1, QM):
                qk_group(wk_sb, kT_sb, m)
                qk_group(wq_sb, qT_sb, m)

            # ---- attention ---------------------------------------------------
            att_sb = []

            def emit_gather_half(s):
                """DMA oT half -> ag_in, AllGather within the batch group,
                prefetch gathered att^T rows into SBUF (half-columns)."""
                for m in range(QM):
                    nc.sync.dma_start(
                        ag_in_h[s][128 * m : 128 * (m + 1), :],
                        oT_sb[:, m, HTOK * s : HTOK * (s + 1)],
                    )
                nc.gpsimd.collective_compute(
                    "AllGather",
                    mybir.AluOpType.bypass,
                    replica_groups=[[0, 1, 2, 3], [4, 5, 6, 7]],
                    ins=[ag_in_h[s][:]],
                    outs=[ag_out_h[s][:]],
                )
                first = not att_sb
                for k in range(FC):
                    if first:
                        att_sb.append(
                            big.tile([128, tok], F32R, name=f"att{k}", tag="big")
                        )
                    nc.sync.dma_start(
                        att_sb[k][:, HTOK * s : HTOK * (s + 1)],
                        ag_out_h[s][128 * k : 128 * (k + 1), :].bitcast(F32R),
                    )

            for qc in range(NQC):
                o_all = [
                    psum.tile([128, 512], F32, name=f"ops{h}", tag="o", bufs=4)
                    for h in range(2 * QM)
                ]
                for p in range(QM):
                    o_ps = o_all[2 * p : 2 * p + 2]
                    for j in range(TT):
                        sim2 = [
                            psum.tile([128, 512], F32, name=f"sim{hh}",
                                      tag="sim", bufs=4)
                            for hh in range(2)
                        ]
                        ex = stage.tile([128, 1024], F32R, name="ex", tag="ex")
                        for hh in range(2):
                            nc.tensor.matmul(
                                sim2[hh][:, :QCH],
                                kT_sb[64 * hh : 64 * hh + 64, p,
                                      128 * j : 128 * (j + 1)],
                                qT_sb[64 * hh : 64 * hh + 64, p,
                                      QCH * qc : QCH * (qc + 1)],
                                start=True, stop=True,
                            )
                        for hh in range(2):
                            nc.scalar.activation(
                                ex[:, 512 * hh : 512 * hh + QCH],
                                sim2[hh][:, :QCH],
                                mybir.ActivationFunctionType.Exp,
                            )
                        for hh in range(2):
                            h = 2 * p + hh
                            nc.tensor.matmul(
                                o_ps[hh][: DH + 1, :QCH],
                                v_sb[:, j, 65 * h : 65 * h + DH + 1],
                                ex[:, 512 * hh : 512 * hh + QCH],
                                start=(j == 0),
                                stop=(j == TT - 1),
                            )
                for p in range(QM):
                    for hh in range(2):
                        h = 2 * p + hh
                        r = stage.tile([1, QCH], F32R, name="r", tag="r", bufs=2)
                        with nc.allow_low_precision(reason="f32r recip for bcast mm"):
                            nc.vector.reciprocal(r[:], o_all[h][DH : DH + 1, :QCH])
                        rb_ps = psum.tile([128, 512], F32, name="rbps", tag="sim",
                                          bufs=4)
                        nc.tensor.matmul(
                            rb_ps[:64, :QCH], ones_sb[:, :64], r[:],
                            start=True, stop=True,
                        )
                        rb = stage.tile([64, QCH], F32, name="rb", tag="rb", bufs=2)
                        nc.vector.tensor_copy(rb[:], rb_ps[:64, :QCH])
                        nc.vector.tensor_mul(
                            oT_sb[64 * hh : 64 * hh + 64, p,
                                  QCH * qc : QCH * (qc + 1)],
                            o_all[h][:DH, :QCH],
                            rb[:],
                        )
                if SPLIT == 2 and qc == NQC // 2 - 1:
                    emit_gather_half(0)
            # ---- final gather half ------------------------------------------
            emit_gather_half(SPLIT - 1)

            # ---- out projection: out[:, shard] = att @ wout -----------------
            for t in range(TT):
                ps = psum.tile([128, 512], F32, name="outps", tag="sim", bufs=4)
                for k in range(FC):
                    nc.tensor.matmul(
                        ps[:, :NOUT],
                        att_sb[k][:, 128 * t : 128 * (t + 1)],
                        wout_sb[:, k, :],
                        start=(k == 0),
                        stop=(k == FC - 1),
                    )
                os_ = stage.tile([128, NOUT], F32, name="os", tag="os", bufs=2)
                nc.vector.tensor_copy(os_[:], ps[:, :NOUT])
                nc.sync.dma_start(out[128 * t : 128 * (t + 1), :], os_[:])

    if split_waits:
        _split_excess_waits(nc)
    return nc


_NC_CACHE = {}


def _get_nc(dim, tok):
    key = (dim, tok)
    if key not in _NC_CACHE:
        _NC_CACHE[key] = build_nc(dim, tok)
    return _NC_CACHE[key]


def make_in_maps(x, w_qkv, w_out):
    """Host-side sharding/layout prep. x:[2,n,dim] w_qkv:[dim,3*h*dh] w_out:[h*dh,dim]."""
    b, n, dim = x.shape
    scale = DH ** -0.5
    hpc = HEADS_TOTAL // (N_CORES // 2)
    in_maps = []
    xT_b = [np.ascontiguousarray(x[bb].T) for bb in range(b)]
    nout = w_out.shape[1] // 4
    for c in range(N_CORES):
        bb, i = divmod(c, N_CORES // 2)
        heads = range(hpc * i, hpc * (i + 1))
        wq = np.concatenate(
            [w_qkv[:, DH * h : DH * (h + 1)] for h in heads], axis=1
        ) * np.float32(scale)
        wk = np.concatenate(
            [w_qkv[:, FEAT + DH * h : FEAT + DH * (h + 1)] for h in heads], axis=1
        )
        vw = hpc * (DH + 1)
        wv = np.zeros((dim, vw), np.float32)
        wvi = np.zeros((1, vw), np.float32)
        for j, h in enumerate(heads):
            wv[:, 65 * j : 65 * j + DH] = w_qkv[
                :, 2 * FEAT + DH * h : 2 * FEAT + DH * (h + 1)
            ]
            wvi[0, 65 * j + DH] = 1.0
        wo = np.ascontiguousarray(w_out[:, nout * i : nout * (i + 1)])
        in_maps.append({
            "xT": xT_b[bb],
            "wq": np.ascontiguousarray(wq, dtype=np.float32),
            "wk": np.ascontiguousarray(wk, dtype=np.float32),
            "wv": wv,
            "wvi": wvi,
            "ones": np.ones((1, 128), np.float32),
            "wout": wo.astype(np.float32),
        })
    return in_maps


def kernel(x, w_qkv, w_out):
    x = np.asarray(x, np.float32)
    w_qkv = np.asarray(w_qkv, np.float32)
    w_out = np.asarray(w_out, np.float32)
    b, n, dim = x.shape
    nc = _get_nc(dim, n)
    in_maps = make_in_maps(x, w_qkv, w_out)
    res = bass_utils.run_bass_kernel_spmd(nc, in_maps, core_ids=list(range(N_CORES)))
    dout = w_out.shape[1]
    nout = dout // 4
    full = np.empty((b, n, dout), np.float32)
    for c in range(N_CORES):
        bb, i = divmod(c, N_CORES // 2)
        full[bb, :, nout * i : nout * (i + 1)] = res.results[c]["out"]
    return full
